# revision 8
# baseline (speedup 1.0000x reference)
"""ODE-RNN Trainium2 kernel (v2: weights-stationary, all-bf16).

Problem: out[b, t*8+i, :] = 2-layer GRU (H=1024) run over the batch dim
(64 steps) of sequence t (30 sequences), with initial hiddens taken from an
RK4-integrated ODE trajectory (8 grid points, shared across all runs).

Strategy (8 NeuronCores, pure data-parallel, no collectives):
  - ODE trajectory on host (f32, <1.2% of FLOPs, latency-serial).
  - Core i handles the 30 runs with init traj[i] (weights replicated).
  - All tensors transposed: state h kept as [128 h-part, KC, 30 runs]; the
    recurrent matmuls use the WEIGHTS as the PE-stationary operand and the
    state as the moving operand, so each gate matmul streams only 30 moving
    rows ([128 gates, 30 runs] psum out) instead of 512 weight columns.
  - Everything bf16 (1 cycle/row at any size); psum accumulation f32.
  - Phase A (dense): gi1 = wi0 @ xT for all 64*32 cols -> DRAM (bf16),
    biases (bi+bh for r/z, bi for n) folded in at PSUM evacuation.
  - Fused recurrence loop: per step, layer-1 gates (gh mm + gi ident-add),
    h1 update (DVE/ACT, transposed layout, full 128 partitions), then
    layer-2 gi2/gh2 both computed on the PE, h2 update straight into the
    output staging tile. No PE transposes anywhere.
  - Host reassembles [128,KC,steps,30] bf16 per-core outputs into (B,240,H).
"""

import numpy as np

try:
    import concourse.bass as bass  # noqa: F401
except ImportError:  # pragma: no cover - fallback for bare environments
    import sys
    sys.path.insert(0, "/opt/trn_rl_repo")
    import concourse.bass as bass  # noqa: F401

import ml_dtypes
import concourse.mybir as mybir
import concourse.tile as tile
from concourse import bacc
from concourse.bass_utils import run_bass_kernel_spmd
from concourse.masks import make_identity

F32 = mybir.dt.float32
BF16 = mybir.dt.bfloat16
AF = mybir.ActivationFunctionType

H = 1024        # hidden size
G3 = 3 * H      # gate width
KC = H // 128   # K chunks (8)
NCH = G3 // 128  # gate chunks (24)
T = 30          # sequences
RN = 30         # real runs per core
R = 32          # padded run count in DRAM layouts
NSEG = 8
SUB = 4
NCORES = 8
BFNP = ml_dtypes.bfloat16


def build_nc(steps=64):
    """Build the per-core Bass module (same program on all 8 cores)."""
    MT = steps * R            # gi col count (2048 for steps=64)
    MB = MT // 512            # phase-A col blocks (4)
    NB = steps // 8           # 8-step blocks (8)
    nc = bacc.Bacc()

    xtr = nc.declare_dram_parameter("xtr", [128, KC, MT], BF16, isOutput=False)
    wi0t = nc.declare_dram_parameter("wi0t", [H, G3], BF16, isOutput=False)
    wh0t = nc.declare_dram_parameter("wh0t", [H, G3], BF16, isOutput=False)
    wi1t = nc.declare_dram_parameter("wi1t", [H, G3], BF16, isOutput=False)
    wh1t = nc.declare_dram_parameter("wh1t", [H, G3], BF16, isOutput=False)
    biasA = nc.declare_dram_parameter("biasA", [128, NCH], F32, isOutput=False)
    biasT = nc.declare_dram_parameter("biasT", [128, 40, RN], BF16,
                                      isOutput=False)
    h1t0 = nc.declare_dram_parameter("h1t0", [128, KC, RN], BF16,
                                     isOutput=False)
    h2t0 = nc.declare_dram_parameter("h2t0", [128, KC, RN], BF16,
                                     isOutput=False)
    out = nc.declare_dram_parameter("out", [128, KC, steps, RN], BF16,
                                    isOutput=True)

    gi1t = nc.dram_tensor("gi1t", [128, NCH, steps, R], BF16)

    def re3(ap, c=KC):
        return ap.rearrange("p (c r) -> p c r", c=c)

    with tile.TileContext(nc) as tc:
        with (
            tc.tile_pool(name="wp", bufs=1) as wp,
            tc.tile_pool(name="cn", bufs=1) as cn,
        ):
            def load_w(param, label):
                wt = wp.tile([128, KC, G3], BF16, tag=f"w{label}",
                             name=f"w_{label}")
                for k in range(KC):
                    nc.sync.dma_start(out=wt[:, k, :],
                                      in_=param[k * 128:(k + 1) * 128, :])
                return wt

            # --- small constants first on the DMA queue ---
            biasAs = cn.tile([128, NCH], F32, tag="biasA", name="biasAs")
            nc.sync.dma_start(out=biasAs, in_=biasA[:])
            biasTs = cn.tile([128, 40, RN], BF16, tag="biasT", name="biasTs")
            nc.sync.dma_start(out=biasTs, in_=biasT[:])
            identf = cn.tile([128, 128], F32, tag="idf", name="identf")
            make_identity(nc, identf)
            identb = cn.tile([128, 128], BF16, tag="idb", name="identb")
            nc.vector.tensor_copy(identb, identf)

            with (
                tc.tile_pool(name="pa", bufs=1) as pa,
                tc.tile_pool(name="w0p", bufs=8) as w0p,
            ):
                # wi0 streamed per gate chunk (8-deep prefetch), so the
                # three recurrence weights can stay SBUF-resident.
                wi0v = wi0t.rearrange("(k p) g -> p k g", k=KC)
                w0tiles = {}

                def w0_dma(c):
                    wc = w0p.tile([128, KC, 128], BF16, tag="w0",
                                  name=f"w0_{c}")
                    nc.sync.dma_start(out=wc,
                                      in_=wi0v[:, :, c * 128:(c + 1) * 128])
                    w0tiles[c] = wc

                for c in range(8):
                    w0_dma(c)
                xtr_s = pa.tile([128, KC, MT], BF16, tag="xtr", name="xtr_s")
                for m in range(MB):
                    nc.sync.dma_start(
                        out=xtr_s[:, :, m * 512:(m + 1) * 512],
                        in_=xtr[:, :, m * 512:(m + 1) * 512])

                # --- recurrence weights stream during phase A ---
                wh0 = load_w(wh0t, "h0")
                wi1 = load_w(wi1t, "i1")
                wh1 = load_w(wh1t, "h1")

                # --- phase A: gi1 = wi0 @ xT (+bias at evacuation) ---
                with (
                    tc.tile_pool(name="psA", bufs=4, space="PSUM") as psA,
                    tc.tile_pool(name="evA", bufs=4) as evA,
                ):
                    for c in range(NCH):
                        if c + 8 < NCH:
                            w0_dma(c + 8)
                        wc = w0tiles.pop(c)
                        for m in range(MB):
                            ps = psA.tile([128, 512], F32, tag="ps",
                                          name=f"psA_{m}_{c}")
                            for k in range(KC):
                                nc.tensor.matmul(
                                    ps, wc[:, k, :],
                                    xtr_s[:, k, m * 512:(m + 1) * 512],
                                    start=(k == 0), stop=(k == KC - 1))
                            ev = evA.tile([128, 512], BF16, tag="ev",
                                          name=f"evA_{m}_{c}")
                            nc.scalar.activation(ev, ps, AF.Identity,
                                                 bias=biasAs[:, c:c + 1])
                            nc.sync.dma_start(
                                out=gi1t[:, c, m * 16:(m + 1) * 16, :],
                                in_=ev.rearrange("p (s r) -> p s r", s=16))

            # ---------------- fused 2-layer recurrence ----------------
            with (
                tc.tile_pool(name="st", bufs=2) as st,
                tc.tile_pool(name="gp", bufs=2) as gp,
                tc.tile_pool(name="tp", bufs=2) as tp,
                tc.tile_pool(name="og", bufs=2) as og,
                tc.tile_pool(name="p1p", bufs=2, space="PSUM") as p1p,
                tc.tile_pool(name="p2p", bufs=2, space="PSUM") as p2p,
            ):
                h1c = st.tile([128, KC, RN], BF16, tag="h1", name="h1_init")
                nc.sync.dma_start(out=h1c, in_=h1t0[:])
                og_cur = og.tile([128, KC, 8, RN], BF16, tag="og",
                                 name="og_init")
                nc.sync.dma_start(out=og_cur[:, :, 7, :], in_=h2t0[:])
                h2v = og_cur[:, :, 7, :]

                gib_cur = gp.tile([128, NCH, 8, R], BF16, tag="gib",
                                  name="gib_0")
                nc.sync.dma_start(out=gib_cur, in_=gi1t[:, :, 0:8, :])
                gib_next = gp.tile([128, NCH, 8, R], BF16, tag="gib",
                                   name="gib_1")
                nc.sync.dma_start(out=gib_next, in_=gi1t[:, :, 8:16, :])

                for s in range(steps):
                    sub = s % 8
                    if sub == 0 and s > 0:
                        gib_cur = gib_next
                        og_cur = og.tile([128, KC, 8, RN], BF16, tag="og",
                                         name=f"og_{s}")
                        if s + 8 < steps:
                            gib_next = gp.tile([128, NCH, 8, R], BF16,
                                               tag="gib", name=f"gib_{s}")
                            nc.sync.dma_start(
                                out=gib_next,
                                in_=gi1t[:, :, s + 8:s + 16, :])
                    p1 = p1p.tile([128, 752], F32, tag="p1", name=f"p1_{s}")
                    p2 = p2p.tile([128, 1008], F32, tag="p2", name=f"p2_{s}")

                    # PSUM start/stop is per 2KB bank ("zero region"): start
                    # on a bank's FIRST matmul pending-zeroes the whole bank
                    # (each later region is zero-initialized on first touch),
                    # stop on its LAST matmul closes the group.
                    # --- L1 matmuls: ghn1 (bank 1) then rz1 (bank 0) ---
                    for c in range(KC):
                        reg = p1[:, 512 + c * RN:512 + (c + 1) * RN]
                        for k in range(KC):
                            nc.tensor.matmul(
                                reg,
                                wh0[:, k, 2 * H + c * 128:2 * H + (c + 1) * 128],
                                h1c[:, k, :], start=(c == 0 and k == 0),
                                stop=False)
                        nc.tensor.matmul(reg, identb, biasTs[:, 16 + c, :],
                                         start=False, stop=(c == KC - 1))
                    for j in range(16):
                        reg = p1[:, j * RN:(j + 1) * RN]
                        for k in range(KC):
                            nc.tensor.matmul(
                                reg, wh0[:, k, j * 128:(j + 1) * 128],
                                h1c[:, k, :], start=(j == 0 and k == 0),
                                stop=False)
                        nc.tensor.matmul(reg, identb,
                                         gib_cur[:, j, sub, 0:RN],
                                         start=False, stop=(j == 15))

                    # --- L2 gh-part matmuls (depend on h2 of step s-1) ---
                    # ghn2 opens p2 bank 1 (shared with gin2, closed there).
                    for c in range(KC):
                        reg = p2[:, 512 + c * RN:512 + (c + 1) * RN]
                        for k in range(KC):
                            nc.tensor.matmul(
                                reg,
                                wh1[:, k, 2 * H + c * 128:2 * H + (c + 1) * 128],
                                h2v[:, k, :], start=(c == 0 and k == 0),
                                stop=False)
                        nc.tensor.matmul(reg, identb, biasTs[:, 24 + c, :],
                                         start=False, stop=False)
                    for j in range(16):
                        reg = p2[:, j * RN:(j + 1) * RN]
                        for k in range(KC):
                            nc.tensor.matmul(
                                reg, wh1[:, k, j * 128:(j + 1) * 128],
                                h2v[:, k, :], start=(j == 0 and k == 0),
                                stop=False)

                    # --- L1 elementwise chain (ACT/DVE) ---
                    def tt(nm):
                        return tp.tile([128, KC, RN], BF16, tag=nm,
                                       name=f"{nm}_{s}")
                    sig1 = tp.tile([128, 16, RN], BF16, tag="sig1",
                                   name=f"sig1_{s}")
                    nc.scalar.activation(sig1, re3(p1[:, 0:480], c=16),
                                         AF.Sigmoid)
                    t1 = tt("t1")
                    nc.vector.tensor_mul(t1, sig1[:, 0:8, :],
                                         re3(p1[:, 512:752]))
                    t2 = tt("t2")
                    nc.vector.tensor_add(t2, t1, gib_cur[:, 16:24, sub, 0:RN])
                    n1 = tt("n1")
                    nc.scalar.activation(n1, t2, AF.Tanh)
                    d1 = tt("d1")
                    nc.vector.tensor_sub(d1, h1c, n1)
                    e1 = tt("e1")
                    nc.vector.tensor_mul(e1, sig1[:, 8:16, :], d1)
                    h1n = st.tile([128, KC, RN], BF16, tag="h1",
                                  name=f"h1_{s}")
                    nc.vector.tensor_add(h1n, n1, e1)

                    # --- L2 gi-part matmuls (depend on h1n) ---
                    for j in range(16):
                        reg = p2[:, j * RN:(j + 1) * RN]
                        for k in range(KC):
                            nc.tensor.matmul(
                                reg, wi1[:, k, j * 128:(j + 1) * 128],
                                h1n[:, k, :], start=False, stop=False)
                        nc.tensor.matmul(reg, identb, biasTs[:, j, :],
                                         start=False, stop=(j == 15))
                    for c in range(KC):
                        reg = p2[:, 768 + c * RN:768 + (c + 1) * RN]
                        for k in range(KC):
                            nc.tensor.matmul(
                                reg,
                                wi1[:, k, 2 * H + c * 128:2 * H + (c + 1) * 128],
                                h1n[:, k, :], start=False, stop=False)
                        nc.tensor.matmul(reg, identb, biasTs[:, 32 + c, :],
                                         start=False, stop=(c == KC - 1))

                    # --- L2 elementwise chain ---
                    sig2 = tp.tile([128, 16, RN], BF16, tag="sig2",
                                   name=f"sig2_{s}")
                    nc.scalar.activation(sig2, re3(p2[:, 0:480], c=16),
                                         AF.Sigmoid)
                    t1b = tt("t1b")
                    nc.vector.tensor_mul(t1b, sig2[:, 0:8, :],
                                         re3(p2[:, 512:752]))
                    t2b = tt("t2b")
                    nc.vector.tensor_add(t2b, t1b, re3(p2[:, 768:1008]))
                    n2 = tt("n2")
                    nc.scalar.activation(n2, t2b, AF.Tanh)
                    d2 = tt("d2")
                    nc.vector.tensor_sub(d2, h2v, n2)
                    e2 = tt("e2")
                    nc.vector.tensor_mul(e2, sig2[:, 8:16, :], d2)
                    h2v = og_cur[:, :, sub, :]
                    nc.vector.tensor_add(h2v, n2, e2)

                    h1c = h1n
                    if sub == 7:
                        nc.sync.dma_start(
                            out=out[:, :, s - 7:s + 1, :], in_=og_cur)

    nc.finalize()
    return nc


def ode_traj(w1, b1, w2, b2, w3, b3):
    """RK4 trajectory of the ODE, mirroring the reference exactly (fp32)."""
    w1t = w1.T.astype(np.float32)
    w2t = w2.T.astype(np.float32)
    w3t = w3.T.astype(np.float32)

    def f(h):
        a = np.tanh(h @ w1t + b1)
        a = np.tanh(a @ w2t + b2)
        return a @ w3t + b3

    dt = np.float32((1.0 / NSEG) / SUB)
    h = np.zeros((2, H), np.float32)
    traj = []
    for _ in range(NSEG):
        for _ in range(SUB):
            k1 = f(h)
            k2 = f(h + np.float32(0.5) * dt * k1)
            k3 = f(h + np.float32(0.5) * dt * k2)
            k4 = f(h + dt * k3)
            h = h + (dt / np.float32(6.0)) * (k1 + np.float32(2.0) * k2
                                              + np.float32(2.0) * k3 + k4)
        traj.append(h.copy())
    return np.stack(traj)  # (NSEG, 2, H)


def _bf(a):
    return np.ascontiguousarray(a).astype(BFNP)


def make_in_maps(x, w1, b1, w2, b2, w3, b3, wi0, wh0, bi0, bh0,
                 wi1, wh1, bi1, bh1, steps=64, cores=NCORES):
    traj = ode_traj(w1, b1, w2, b2, w3, b3)
    MT = steps * R

    # xtr[p, k, s*R + r] = x[s, r, k*128+p]
    xp = np.zeros((steps, R, H), np.float32)
    xp[:, :T, :] = x[:steps, :, :]
    xtr = xp.reshape(MT, KC, 128).transpose(2, 1, 0)

    brz0 = np.concatenate([bi0[:2 * H] + bh0[:2 * H], bi0[2 * H:]])
    biasA = np.ascontiguousarray(
        brz0.reshape(NCH, 128).T.astype(np.float32))
    bt = np.concatenate([bi1[:2 * H] + bh1[:2 * H],   # L2 rz (16 chunks)
                         bh0[2 * H:],                 # L1 ghn bias (8)
                         bh1[2 * H:],                 # L2 ghn bias (8)
                         bi1[2 * H:]])                # L2 gin bias (8)
    biasT = np.broadcast_to(
        bt.reshape(40, 128).T.astype(np.float32)[:, :, None], (128, 40, RN))

    shared = {
        "xtr": _bf(xtr),
        "wi0t": _bf(wi0.T), "wh0t": _bf(wh0.T),
        "wi1t": _bf(wi1.T), "wh1t": _bf(wh1.T),
        "biasA": biasA,
        "biasT": _bf(biasT),
    }
    in_maps = []
    for i in range(cores):
        m = dict(shared)
        for nm, h in (("h1t0", traj[i, 0]), ("h2t0", traj[i, 1])):
            ht = np.broadcast_to(
                h.reshape(KC, 128).T.astype(np.float32)[:, :, None],
                (128, KC, RN))
            m[nm] = _bf(ht)
        in_maps.append(m)
    return in_maps


_NC_CACHE = {}


def _get_nc(steps):
    if steps not in _NC_CACHE:
        _NC_CACHE[steps] = build_nc(steps)
    return _NC_CACHE[steps]


def run_cores(inputs, steps=64, cores=NCORES, **run_kwargs):
    in_maps = make_in_maps(steps=steps, cores=cores, **inputs)
    nc = _get_nc(steps)
    return run_bass_kernel_spmd(nc, in_maps, core_ids=list(range(cores)),
                                **run_kwargs)


def kernel(x, w1, b1, w2, b2, w3, b3, wi0, wh0, bi0, bh0,
           wi1, wh1, bi1, bh1):
    args = dict(x=x, w1=w1, b1=b1, w2=w2, b2=b2, w3=w3, b3=b3,
                wi0=wi0, wh0=wh0, bi0=bi0, bh0=bh0,
                wi1=wi1, wh1=wh1, bi1=bi1, bh1=bh1)
    args = {k: np.asarray(v, np.float32) for k, v in args.items()}
    res = run_cores(args, steps=64, cores=NCORES)
    B = 64
    full = np.empty((B, T * NCORES, H), np.float32)
    for i in range(NCORES):
        o = np.asarray(res.results[i]["out"]).astype(np.float32)
        full[:, i::NCORES, :] = o.transpose(2, 3, 1, 0).reshape(B, RN, H)
    return full
